# revision 5
# baseline (speedup 1.0000x reference)
"""Trainium2 Bass kernel: GCN layer + global-attention pooling + MLP head.

Contract: kernel(**inputs) takes the FULL (unsharded) numpy inputs of
reference.setup_inputs() and returns the FULL output tuple
(sigmoid_logits [G,2], att [N,1], hg [G,H]) — same structure as reference().

Sharding strategy (8 NeuronCores): nodes and their incoming edges are
partitioned by graph (graph_id is sorted; 8 graphs per core), so per-graph
softmax pooling is core-local.  h is replicated; each core gathers the
source rows for its own edges directly from DRAM with dma_gather and
scatter-adds them into PSUM via scaled one-hot matmuls on the PE.

Device math (W_conv commuted past the linear aggregation):
    agg_raw = segment_sum(rsqrt(deg_out)[src] * h[src] -> dst)
    hr      = relu((agg_raw * rsqrt(deg_in)) @ W_conv + b_conv)
    e       = exp(hr @ w_gate + b_gate)          (stable: gates are O(1))
    hg      = (one_hot(graph)ᵀ·e·[hr|1]) ;  att = e / denom[graph]
    out     = sigmoid((hg @ W1 + b1) @ W2 + b2)

Host preprocessing is integer graph restructuring only (bucketing edges by
destination tile, padding, degree counts); all float math runs on device.
"""

import heapq

import numpy as np

P = 128
HID = 256
NCLS = 2
N_CORES = 8

_CACHE = {}


# ---------------------------------------------------------------------------
# Host-side preprocessing (integer-only: sharding, bucketing, padding)
# ---------------------------------------------------------------------------

def _prep(inputs, n_cores=N_CORES):
    h = np.ascontiguousarray(np.asarray(inputs["h"], dtype=np.float32))
    src = np.asarray(inputs["src"]).astype(np.int64)
    dst = np.asarray(inputs["dst"]).astype(np.int64)
    gid = np.asarray(inputs["graph_id"]).astype(np.int64)
    N = h.shape[0]
    E = src.shape[0]
    G = max(int(gid.max()) + 1 if gid.size else 0, n_cores)
    gpc = (G + n_cores - 1) // n_cores
    assert gpc * n_cores == G and gpc == 8, (G, n_cores)

    deg_out = np.bincount(src, minlength=N)
    deg_in = np.bincount(dst, minlength=N)
    gcounts = np.bincount(gid, minlength=G)
    gstart = np.concatenate([[0], np.cumsum(gcounts)])

    core_lo = gstart[np.arange(n_cores) * gpc]
    core_hi = gstart[np.minimum((np.arange(n_cores) + 1) * gpc, G)]
    counts = core_hi - core_lo
    TN = int(-(-counts.max() // P))

    # Balanced node->(tile,partition) packing per core: place nodes (desc by
    # in-degree) into the least-loaded of TN bins with <128 nodes, so
    # per-tile edge counts are nearly equal -> minimal chunk count K.
    node_tile = np.full(N, -1, np.int64)
    node_part = np.full(N, -1, np.int64)
    slot_node = np.full((n_cores, TN, P), -1, np.int64)
    for c in range(n_cores):
        nodes = np.arange(core_lo[c], core_hi[c])
        order = nodes[np.argsort(-deg_in[nodes], kind="stable")]
        heap = [(0, 0, t) for t in range(TN)]
        heapq.heapify(heap)
        for n in order:
            load, cnt, t = heapq.heappop(heap)
            node_tile[n] = t
            node_part[n] = cnt
            slot_node[c, t, cnt] = n
            cnt += 1
            if cnt < P:
                heapq.heappush(heap, (load + int(deg_in[n]), cnt, t))

    node_core = np.searchsorted(core_hi, np.arange(N), side="right")
    e_core = node_core[dst]
    e_tile = node_tile[dst]
    tile_cnt = np.zeros((n_cores, TN), np.int64)
    np.add.at(tile_cnt, (e_core, e_tile), 1)
    K = int(-(-tile_cnt.max() // P))
    CE = TN * K
    EPC = CE * P

    srcpad = np.zeros((n_cores, EPC), np.int64)
    dstl = np.full((n_cores, EPC), 999.0, np.float32)
    degsrc = np.ones((n_cores, EPC), np.float32)
    order = np.lexsort((e_tile, e_core))
    eo = np.arange(E)[order]
    ec, et = e_core[order], e_tile[order]
    grp = ec * TN + et
    pos = np.arange(E) - np.concatenate(
        [[0], np.cumsum(np.bincount(grp, minlength=n_cores * TN))])[grp]
    slot = et * (K * P) + pos
    srcpad[ec, slot] = src[eo]
    dstl[ec, slot] = node_part[dst[eo]].astype(np.float32)
    degsrc[ec, slot] = np.maximum(deg_out[src[eo]], 1).astype(np.float32)

    # wrapped int16 gather indices: idx j -> partition j%16, col j//16,
    # replicated across the 8 groups of 16 partitions.
    assert N <= 32767
    idx16 = srcpad.reshape(n_cores, EPC // 16, 16).transpose(0, 2, 1)
    idx16 = np.tile(idx16.astype(np.int16), (1, 8, 1))
    dstl_t = dstl.reshape(n_cores, CE, P).transpose(0, 2, 1).copy()
    degsrc_t = degsrc.reshape(n_cores, CE, P).transpose(0, 2, 1).copy()

    degin_t = np.ones((n_cores, P, TN), np.float32)
    gidl_t = np.full((n_cores, P, TN), 999.0, np.float32)
    for c in range(n_cores):
        sn = slot_node[c]
        real = sn >= 0
        nn = sn[real]
        degin_t[c].T[real] = np.maximum(deg_in[nn], 1).astype(np.float32)
        gidl_t[c].T[real] = (gid[nn] - c * gpc).astype(np.float32)
    gidl8 = gidl_t.transpose(0, 2, 1).reshape(n_cores, 1, TN * P)
    gidl8 = np.broadcast_to(gidl8, (n_cores, 8, TN * P)).copy()

    W_conv = np.asarray(inputs["W_conv"], np.float32)
    b_conv = np.asarray(inputs["b_conv"], np.float32)
    w_gate = np.asarray(inputs["w_gate"], np.float32).reshape(-1)
    b_gate = np.asarray(inputs["b_gate"], np.float32).reshape(-1)
    W1 = np.asarray(inputs["W1"], np.float32)
    b1 = np.asarray(inputs["b1"], np.float32)
    W2 = np.asarray(inputs["W2"], np.float32)
    b2 = np.asarray(inputs["b2"], np.float32)

    consts = dict(
        iotaK=np.ascontiguousarray(np.broadcast_to(
            np.tile(np.arange(P, dtype=np.float32), K), (P, K * P))),
        w_conv=np.ascontiguousarray(W_conv),
        w1=np.ascontiguousarray(W1),
        w2=np.ascontiguousarray(W2),
        bconv_bc=np.broadcast_to(b_conv, (P, HID)).copy(),
        wg_bc=np.broadcast_to(w_gate, (P, HID)).copy(),
        bgate_col=np.full((P, 1), b_gate[0], np.float32),
        b1_bc8=np.broadcast_to(b1, (8, HID)).copy(),
        b2_bc8=np.broadcast_to(b2, (8, NCLS)).copy(),
        iota128=np.broadcast_to(np.arange(P, dtype=np.float32), (P, P)).copy(),
        iota8=np.broadcast_to(np.arange(8, dtype=np.float32), (P, 8)).copy(),
        iota_col8=np.arange(8, dtype=np.float32).reshape(8, 1),
        ident=np.eye(P, dtype=np.float32),
    )

    in_maps = []
    for c in range(n_cores):
        m = dict(consts)
        m["h"] = h
        m["idx16"] = np.ascontiguousarray(idx16[c])
        m["dstl"] = np.ascontiguousarray(dstl_t[c])
        m["degsrc"] = np.ascontiguousarray(degsrc_t[c])
        m["degin"] = np.ascontiguousarray(degin_t[c])
        m["gidl"] = np.ascontiguousarray(gidl_t[c])
        m["gidl8"] = np.ascontiguousarray(gidl8[c])
        in_maps.append(m)

    meta = dict(N=N, E=E, G=G, gpc=gpc, TN=TN, K=K, CE=CE, EPC=EPC,
                n_cores=n_cores, slot_node=slot_node)
    return in_maps, meta


# ---------------------------------------------------------------------------
# Device program
# ---------------------------------------------------------------------------

def _build_nc(meta, n_cores):
    import concourse.bacc as bacc
    import concourse.mybir as mybir
    import concourse.tile as tile

    dt = mybir.dt
    op = mybir.AluOpType
    act = mybir.ActivationFunctionType
    TN, K, CE, EPC = meta["TN"], meta["K"], meta["CE"], meta["EPC"]
    N = meta["N"]

    nc = bacc.Bacc("TRN2", target_bir_lowering=False, debug=False,
                   num_devices=n_cores, num_swdge_queues=4)

    def din(name, shape, d=dt.float32):
        return nc.dram_tensor(name, shape, d, kind="ExternalInput")

    h_d = din("h", [N, HID], dt.float32r)
    idx_d = din("idx16", [P, EPC // 16], dt.int16)
    dstl_d = din("dstl", [P, CE])
    degsrc_d = din("degsrc", [P, CE])
    degin_d = din("degin", [P, TN])
    gidl_d = din("gidl", [P, TN])
    gidl8_d = din("gidl8", [8, TN * P])
    wconv_d = din("w_conv", [HID, HID])
    w1_d = din("w1", [HID, HID])
    w2_d = din("w2", [HID, NCLS])
    bconv_d = din("bconv_bc", [P, HID])
    wg_d = din("wg_bc", [P, HID])
    bgate_d = din("bgate_col", [P, 1])
    b1_d = din("b1_bc8", [8, HID])
    b2_d = din("b2_bc8", [8, NCLS])
    iota128_d = din("iota128", [P, P])
    iotaK_d = din("iotaK", [P, K * P])
    iota8_d = din("iota8", [P, 8])
    iotac8_d = din("iota_col8", [8, 1])
    ident_d = din("ident", [P, P])

    att_o = nc.dram_tensor("att_out", [P, TN], dt.float32, kind="ExternalOutput")
    hg_o = nc.dram_tensor("hg_out", [8, HID], dt.float32, kind="ExternalOutput")
    sig_o = nc.dram_tensor("sig_out", [8, NCLS], dt.float32, kind="ExternalOutput")

    with tile.TileContext(nc) as tc:
        with (
            tc.tile_pool(name="const", bufs=1) as cpool,
            tc.tile_pool(name="meta", bufs=1) as mpool,
            tc.tile_pool(name="gather", bufs=3) as gpool,
            tc.tile_pool(name="oh", bufs=4) as ohpool,
            tc.tile_pool(name="work", bufs=3) as wpool,
            tc.tile_pool(name="psA", bufs=2, space="PSUM") as psA,
            tc.tile_pool(name="psB", bufs=2, space="PSUM") as psB,
            tc.tile_pool(name="psC", bufs=2, space="PSUM") as psC,
            tc.tile_pool(name="psAcc", bufs=1, space="PSUM") as psAcc,
        ):
            def load(pool, dram, shape, d=dt.float32):
                t = pool.tile(shape, d, tag=dram.name)
                nc.sync.dma_start(t[:], dram.ap())
                return t

            def load_w(dram, X, tag):
                t = cpool.tile([P, 2 * X], dt.float32, tag=tag)
                nc.sync.dma_start(t[:, 0:X], dram.ap()[0:P, :])
                nc.sync.dma_start(t[:, X:2 * X], dram.ap()[P:2 * P, :])
                return t

            wconv = load_w(wconv_d, HID, "wconv")
            w1 = load_w(w1_d, HID, "w1")
            w2 = load_w(w2_d, NCLS, "w2")
            bconv = load(cpool, bconv_d, [P, HID])
            wg = load(cpool, wg_d, [P, HID])
            bgate = load(cpool, bgate_d, [P, 1])
            b1b = load(cpool, b1_d, [8, HID])
            b2b = load(cpool, b2_d, [8, NCLS])
            iota128 = load(cpool, iota128_d, [P, P])
            iotaK = load(cpool, iotaK_d, [P, K * P])
            iota8 = load(cpool, iota8_d, [P, 8])
            iotac8 = load(cpool, iotac8_d, [8, 1])
            ident = load(cpool, ident_d, [P, P])
            idx16 = load(mpool, idx_d, [P, EPC // 16], dt.int16)
            dstl = load(mpool, dstl_d, [P, CE])
            degsrc = load(mpool, degsrc_d, [P, CE])
            degin = load(mpool, degin_d, [P, TN])
            gidl = load(mpool, gidl_d, [P, TN])
            gidl8 = load(mpool, gidl8_d, [8, TN * P])

            scol = mpool.tile([P, CE], dt.float32)
            nc.scalar.sqrt(scol[:], degsrc[:])
            nc.vector.reciprocal(scol[:], scol[:])
            rin = mpool.tile([P, TN], dt.float32)
            nc.scalar.sqrt(rin[:], degin[:])
            nc.vector.reciprocal(rin[:], rin[:])

            e_all = mpool.tile([P, TN], dt.float32)
            att_all = mpool.tile([P, TN], dt.float32)
            hg_ps = psAcc.tile([8, HID + 1], dt.float32, tag="hg")

            # dma_gather calls are capped at 1024 indices (SWDGE ring holds
            # 1024 descriptors); batch 8 chunks per call, spanning tiles.
            GB = 8
            msg_tiles = {}

            def get_msg(c):
                j = c // GB
                if j not in msg_tiles:
                    lo = j * GB
                    nch = min(GB, CE - lo)
                    mt = gpool.tile([P, nch, HID], dt.float32r, tag="msg")
                    nc.gpsimd.dma_gather(
                        mt[:], h_d.ap(),
                        idx16[:, lo * 8:(lo + nch) * 8],
                        nch * P, nch * P, HID,
                        queue_num=j % 4,
                    )
                    msg_tiles[j] = mt
                return msg_tiles[j][:, c % GB, :]

            for t in range(TN):
                agg_ps = psA.tile([P, HID], dt.float32, tag="agg")
                # one wide is_equal + one wide multiply build all K scaled
                # one-hot matrices at once (per-chunk tensor_scalar ops with
                # PTR scalars measured 8x slower under DMA contention)
                oh_all = ohpool.tile([P, K, P], dt.float32r, tag="oh")
                dv = dstl[:, t * K:(t + 1) * K].to_broadcast([P, K, P])
                sv = scol[:, t * K:(t + 1) * K].to_broadcast([P, K, P])
                nc.vector.tensor_tensor(
                    out=oh_all[:], in0=iotaK[:].rearrange(
                        "p (k q) -> p k q", k=K), in1=dv, op=op.is_equal)
                nc.vector.tensor_tensor(out=oh_all[:], in0=oh_all[:], in1=sv,
                                        op=op.mult)
                for k in range(K):
                    c = t * K + k
                    nc.tensor.matmul(agg_ps[:], lhsT=oh_all[:, k, :],
                                     rhs=get_msg(c),
                                     start=(k == 0), stop=(k == K - 1))
                agg_sb = wpool.tile([P, HID], dt.float32, tag="agg_sb")
                nc.vector.tensor_scalar(out=agg_sb[:], in0=agg_ps[:],
                                        scalar1=rin[:, t:t + 1], scalar2=None,
                                        op0=op.mult)
                tr_ps = psB.tile([P, HID], dt.float32, tag="tr")
                nc.tensor.transpose(tr_ps[:, 0:P], agg_sb[:, 0:P], ident[:])
                nc.tensor.transpose(tr_ps[:, P:HID], agg_sb[:, P:HID], ident[:])
                aggT = wpool.tile([P, HID], dt.float32, tag="aggT")
                nc.vector.tensor_copy(aggT[:], tr_ps[:])
                hr_ps = psC.tile([P, HID], dt.float32, tag="hr")
                for fh in range(2):
                    nc.tensor.matmul(hr_ps[:],
                                     lhsT=aggT[:, fh * P:(fh + 1) * P],
                                     rhs=wconv[:, fh * HID:(fh + 1) * HID],
                                     start=(fh == 0), stop=(fh == 1))
                hr = wpool.tile([P, HID + 1], dt.float32, tag="hr_sb")
                nc.vector.tensor_add(hr[:, 0:HID], hr_ps[:], bconv[:])
                nc.scalar.activation(hr[:, 0:HID], hr[:, 0:HID], act.Relu)
                nc.vector.memset(hr[:, HID:HID + 1], 1.0)
                ttr = wpool.tile([P, HID], dt.float32, tag="ttr")
                gcol = wpool.tile([P, 1], dt.float32, tag="gcol")
                nc.vector.scalar_tensor_tensor(
                    out=ttr[:], in0=hr[:, 0:HID], scalar=1.0, in1=wg[:],
                    op0=op.mult, op1=op.mult, accum_out=gcol[:],
                )
                nc.scalar.activation(e_all[:, t:t + 1], gcol[:], act.Exp,
                                     bias=bgate[:, 0:1])
                mg = wpool.tile([P, 8], dt.float32, tag="mg")
                nc.vector.tensor_scalar(
                    out=mg[:], in0=iota8[:],
                    scalar1=gidl[:, t:t + 1], scalar2=e_all[:, t:t + 1],
                    op0=op.is_equal, op1=op.mult,
                )
                nc.tensor.matmul(hg_ps[:], lhsT=mg[:], rhs=hr[:],
                                 start=(t == 0), stop=(t == TN - 1))

            dn = wpool.tile([8, 1], dt.float32, tag="dn")
            nc.vector.tensor_scalar(out=dn[:], in0=hg_ps[:, HID:HID + 1],
                                    scalar1=1e-30, scalar2=None, op0=op.max)
            recip = wpool.tile([8, 1], dt.float32, tag="recip")
            nc.vector.reciprocal(recip[:], dn[:])
            hg_sb = wpool.tile([8, HID], dt.float32, tag="hg_sb")
            nc.vector.tensor_scalar(out=hg_sb[:], in0=hg_ps[:, 0:HID],
                                    scalar1=recip[:, 0:1], scalar2=None,
                                    op0=op.mult)
            nc.sync.dma_start(hg_o.ap(), hg_sb[:])

            for t in range(TN):
                mgT = wpool.tile([8, P], dt.float32, tag="mgT")
                nc.vector.tensor_scalar(
                    out=mgT[:], in0=gidl8[:, t * P:(t + 1) * P],
                    scalar1=iotac8[:, 0:1], scalar2=None, op0=op.is_equal,
                )
                rp_ps = psB.tile([P, 1], dt.float32, tag="tr")
                nc.tensor.matmul(rp_ps[:], lhsT=mgT[:], rhs=recip[:],
                                 start=True, stop=True)
                nc.vector.tensor_mul(att_all[:, t:t + 1], e_all[:, t:t + 1],
                                     rp_ps[:])
            nc.sync.dma_start(att_o.ap(), att_all[:])

            def transpose2(src_sb, tag):
                tp = psB.tile([P, 16], dt.float32, tag="tr")
                nc.tensor.transpose(tp[:, 0:8], src_sb[:, 0:P], ident[0:8, 0:8])
                nc.tensor.transpose(tp[:, 8:16], src_sb[:, P:HID],
                                    ident[0:8, 0:8])
                sb = wpool.tile([P, 16], dt.float32, tag=tag)
                nc.vector.tensor_copy(sb[:], tp[:])
                return sb

            hgT = transpose2(hg_sb, "hgT")
            a2_ps = psA.tile([8, HID], dt.float32, tag="agg")
            for fh in range(2):
                nc.tensor.matmul(a2_ps[:], lhsT=hgT[:, fh * 8:(fh + 1) * 8],
                                 rhs=w1[:, fh * HID:(fh + 1) * HID],
                                 start=(fh == 0), stop=(fh == 1))
            a2 = wpool.tile([8, HID], dt.float32, tag="a2")
            nc.vector.tensor_add(a2[:], a2_ps[:], b1b[:])
            a2T = transpose2(a2, "a2T")
            a3_ps = psC.tile([8, NCLS], dt.float32, tag="hr")
            for fh in range(2):
                nc.tensor.matmul(a3_ps[:], lhsT=a2T[:, fh * 8:(fh + 1) * 8],
                                 rhs=w2[:, fh * NCLS:(fh + 1) * NCLS],
                                 start=(fh == 0), stop=(fh == 1))
            sig = wpool.tile([8, NCLS], dt.float32, tag="sig")
            nc.vector.tensor_add(sig[:], a3_ps[:], b2b[:])
            nc.scalar.activation(sig[:], sig[:], act.Sigmoid)
            nc.sync.dma_start(sig_o.ap(), sig[:])

    nc.compile()
    return nc


def _assemble(results, meta):
    n_cores = meta["n_cores"]
    N, G, gpc = meta["N"], meta["G"], meta["gpc"]
    slot_node = meta["slot_node"]
    out0 = np.zeros((G, NCLS), np.float32)
    hg = np.zeros((G, HID), np.float32)
    att = np.zeros((N, 1), np.float32)
    for c in range(n_cores):
        r = results[c]
        out0[c * gpc:(c + 1) * gpc] = r["sig_out"]
        hg[c * gpc:(c + 1) * gpc] = r["hg_out"]
        sn = slot_node[c]
        real = sn >= 0
        att[sn[real], 0] = r["att_out"].T[real]
    return out0, att, hg


TRACE = False          # set True (e.g. from test.py) to capture an NTFF profile
LAST_PERF = {}         # exec_time_ns etc. from the last traced run


def _install_axon_ntff_hook():
    """Best-effort: register the NTFF profile hook concourse expects under
    axon (the agent image's antenv lacks axon_hooks)."""
    import contextlib
    import ctypes
    import sys
    import types
    if 'antenv.axon_hooks' in sys.modules:
        return
    try:
        lib = ctypes.CDLL('/opt/axon/libaxon_pjrt.so')
        lib.axon_start_nrt_profile.argtypes = [ctypes.POINTER(ctypes.c_int64),
                                               ctypes.c_size_t]
        lib.axon_start_nrt_profile.restype = ctypes.c_int64
        lib.axon_stop_nrt_profile.argtypes = [ctypes.c_char_p]
        lib.axon_stop_nrt_profile.restype = ctypes.c_int64
    except (OSError, AttributeError):
        return

    @contextlib.contextmanager
    def _hook(output_dir, device_ids):
        import jax
        jax.devices()
        if device_ids:
            ids = (ctypes.c_int64 * len(device_ids))(*device_ids)
            rc = lib.axon_start_nrt_profile(ids, len(device_ids))
        else:
            rc = lib.axon_start_nrt_profile(None, 0)
        if rc != 0:
            raise RuntimeError(f"axon_start_nrt_profile rc={rc}")
        try:
            yield
        finally:
            lib.axon_stop_nrt_profile(str(output_dir).encode())

    mod = types.ModuleType('antenv.axon_hooks')
    mod.get_axon_ntff_profile_hook = lambda: _hook
    mod.set_axon_ntff_profile_hook = lambda h: None
    sys.modules['antenv.axon_hooks'] = mod


def kernel(**inputs):
    import sys
    if '/opt/trn_rl_repo' not in sys.path:
        sys.path.insert(0, '/opt/trn_rl_repo')
    from concourse import bass_utils
    from concourse.bass_utils import run_bass_kernel_spmd

    in_maps, meta = _prep(inputs, N_CORES)
    key = (meta["N"], meta["E"], meta["TN"], meta["K"])
    nc = _CACHE.get(key)
    if nc is None:
        nc = _build_nc(meta, N_CORES)
        _CACHE[key] = nc
    kwargs = {}
    if TRACE:
        _install_axon_ntff_hook()
        bass_utils.upload_artifacts = lambda d: d
        kwargs = dict(trace=True, trace_cores=list(range(N_CORES)))
    res = run_bass_kernel_spmd(nc, in_maps, core_ids=list(range(N_CORES)),
                               **kwargs)
    if TRACE:
        LAST_PERF.update(
            exec_time_ns=res.exec_time_ns,
            mean_exec_time_ns=res.mean_exec_time_ns,
            max_exec_time_core_id=res.max_exec_time_core_id,
            trace=(res.instructions_and_trace or (None, None))[1],
        )
    return _assemble(res.results, meta)


# revision 6
# speedup vs baseline: 1.2132x; 1.2132x over previous
"""Trainium2 Bass kernel: GCN layer + global-attention pooling + MLP head.

Contract: kernel(**inputs) takes the FULL (unsharded) numpy inputs of
reference.setup_inputs() and returns the FULL output tuple
(sigmoid_logits [G,2], att [N,1], hg [G,H]) — same structure as reference().

Sharding strategy (8 NeuronCores): nodes and their incoming edges are
partitioned by graph (graph_id is sorted; 8 graphs per core), so per-graph
softmax pooling is core-local.  h is replicated; each core gathers the
source rows for its own edges directly from DRAM with dma_gather and
scatter-adds them into PSUM via scaled one-hot matmuls on the PE.

Device math (W_conv commuted past the linear aggregation):
    agg_raw = segment_sum(rsqrt(deg_out)[src] * h[src] -> dst)
    hr      = relu((agg_raw * rsqrt(deg_in)) @ W_conv + b_conv)
    e       = exp(hr @ w_gate + b_gate)          (stable: gates are O(1))
    hg      = (one_hot(graph)ᵀ·e·[hr|1]) ;  att = e / denom[graph]
    out     = sigmoid((hg @ W1 + b1) @ W2 + b2)

Host preprocessing is integer graph restructuring only (bucketing edges by
destination tile, padding, degree counts); all float math runs on device.
"""

import heapq

import numpy as np

P = 128
HID = 256
NCLS = 2
N_CORES = 8

_CACHE = {}


# ---------------------------------------------------------------------------
# Host-side preprocessing (integer-only: sharding, bucketing, padding)
# ---------------------------------------------------------------------------

def _prep(inputs, n_cores=N_CORES):
    h = np.ascontiguousarray(np.asarray(inputs["h"], dtype=np.float32))
    src = np.asarray(inputs["src"]).astype(np.int64)
    dst = np.asarray(inputs["dst"]).astype(np.int64)
    gid = np.asarray(inputs["graph_id"]).astype(np.int64)
    N = h.shape[0]
    E = src.shape[0]
    G = max(int(gid.max()) + 1 if gid.size else 0, n_cores)
    gpc = (G + n_cores - 1) // n_cores
    assert gpc * n_cores == G and gpc == 8, (G, n_cores)

    deg_out = np.bincount(src, minlength=N)
    deg_in = np.bincount(dst, minlength=N)
    gcounts = np.bincount(gid, minlength=G)
    gstart = np.concatenate([[0], np.cumsum(gcounts)])

    core_lo = gstart[np.arange(n_cores) * gpc]
    core_hi = gstart[np.minimum((np.arange(n_cores) + 1) * gpc, G)]
    counts = core_hi - core_lo
    TN = int(-(-counts.max() // P))

    # Balanced node->(tile,partition) packing per core: place nodes (desc by
    # in-degree) into the least-loaded of TN bins with <128 nodes, so
    # per-tile edge counts are nearly equal -> minimal chunk count K.
    node_tile = np.full(N, -1, np.int64)
    node_part = np.full(N, -1, np.int64)
    slot_node = np.full((n_cores, TN, P), -1, np.int64)
    for c in range(n_cores):
        nodes = np.arange(core_lo[c], core_hi[c])
        order = nodes[np.argsort(-deg_in[nodes], kind="stable")]
        heap = [(0, 0, t) for t in range(TN)]
        heapq.heapify(heap)
        for n in order:
            load, cnt, t = heapq.heappop(heap)
            node_tile[n] = t
            node_part[n] = cnt
            slot_node[c, t, cnt] = n
            cnt += 1
            if cnt < P:
                heapq.heappush(heap, (load + int(deg_in[n]), cnt, t))

    node_core = np.searchsorted(core_hi, np.arange(N), side="right")
    e_core = node_core[dst]
    e_tile = node_tile[dst]
    tile_cnt = np.zeros((n_cores, TN), np.int64)
    np.add.at(tile_cnt, (e_core, e_tile), 1)
    K = int(-(-tile_cnt.max() // P))
    CE = TN * K
    EPC = CE * P

    srcpad = np.zeros((n_cores, EPC), np.int64)
    dstl = np.full((n_cores, EPC), 999.0, np.float32)
    degsrc = np.ones((n_cores, EPC), np.float32)
    order = np.lexsort((e_tile, e_core))
    eo = np.arange(E)[order]
    ec, et = e_core[order], e_tile[order]
    grp = ec * TN + et
    pos = np.arange(E) - np.concatenate(
        [[0], np.cumsum(np.bincount(grp, minlength=n_cores * TN))])[grp]
    slot = et * (K * P) + pos
    srcpad[ec, slot] = src[eo]
    dstl[ec, slot] = node_part[dst[eo]].astype(np.float32)
    degsrc[ec, slot] = np.maximum(deg_out[src[eo]], 1).astype(np.float32)

    # wrapped int16 gather indices: idx j -> partition j%16, col j//16,
    # replicated across the 8 groups of 16 partitions.
    assert N <= 32767
    idx16 = srcpad.reshape(n_cores, EPC // 16, 16).transpose(0, 2, 1)
    idx16 = np.tile(idx16.astype(np.int16), (1, 8, 1))
    dstl_t = dstl.reshape(n_cores, CE, P).transpose(0, 2, 1).copy()
    degsrc_t = degsrc.reshape(n_cores, CE, P).transpose(0, 2, 1).copy()

    degin_t = np.ones((n_cores, P, TN), np.float32)
    gidl_t = np.full((n_cores, P, TN), 999.0, np.float32)
    for c in range(n_cores):
        sn = slot_node[c]
        real = sn >= 0
        nn = sn[real]
        degin_t[c].T[real] = np.maximum(deg_in[nn], 1).astype(np.float32)
        gidl_t[c].T[real] = (gid[nn] - c * gpc).astype(np.float32)
    gidl8 = gidl_t.transpose(0, 2, 1).reshape(n_cores, 1, TN * P)
    gidl8 = np.broadcast_to(gidl8, (n_cores, 8, TN * P)).copy()

    W_conv = np.asarray(inputs["W_conv"], np.float32)
    b_conv = np.asarray(inputs["b_conv"], np.float32)
    w_gate = np.asarray(inputs["w_gate"], np.float32).reshape(-1)
    b_gate = np.asarray(inputs["b_gate"], np.float32).reshape(-1)
    W1 = np.asarray(inputs["W1"], np.float32)
    b1 = np.asarray(inputs["b1"], np.float32)
    W2 = np.asarray(inputs["W2"], np.float32)
    b2 = np.asarray(inputs["b2"], np.float32)

    consts = dict(
        iotaK=np.ascontiguousarray(np.broadcast_to(
            np.tile(np.arange(P, dtype=np.float32), K), (P, K * P))),
        w_conv=np.ascontiguousarray(W_conv),
        w1=np.ascontiguousarray(W1),
        w2=np.ascontiguousarray(W2),
        bconv_bc=np.broadcast_to(b_conv, (P, HID)).copy(),
        wg_bc=np.broadcast_to(w_gate, (P, HID)).copy(),
        bgate_col=np.full((P, 1), b_gate[0], np.float32),
        b1_bc8=np.broadcast_to(b1, (8, HID)).copy(),
        b2_bc8=np.broadcast_to(b2, (8, NCLS)).copy(),
        iota128=np.broadcast_to(np.arange(P, dtype=np.float32), (P, P)).copy(),
        iota8=np.broadcast_to(np.arange(8, dtype=np.float32), (P, 8)).copy(),
        iota_col8=np.arange(8, dtype=np.float32).reshape(8, 1),
        ident=np.eye(P, dtype=np.float32),
    )

    in_maps = []
    for c in range(n_cores):
        m = dict(consts)
        m["h"] = h
        m["idx16"] = np.ascontiguousarray(idx16[c])
        m["dstl"] = np.ascontiguousarray(dstl_t[c])
        m["degsrc"] = np.ascontiguousarray(degsrc_t[c])
        m["degin"] = np.ascontiguousarray(degin_t[c])
        m["gidl"] = np.ascontiguousarray(gidl_t[c])
        m["gidl8"] = np.ascontiguousarray(gidl8[c])
        in_maps.append(m)

    meta = dict(N=N, E=E, G=G, gpc=gpc, TN=TN, K=K, CE=CE, EPC=EPC,
                n_cores=n_cores, slot_node=slot_node)
    return in_maps, meta


# ---------------------------------------------------------------------------
# Device program
# ---------------------------------------------------------------------------

def _build_nc(meta, n_cores):
    import concourse.bacc as bacc
    import concourse.mybir as mybir
    import concourse.tile as tile

    dt = mybir.dt
    op = mybir.AluOpType
    act = mybir.ActivationFunctionType
    TN, K, CE, EPC = meta["TN"], meta["K"], meta["CE"], meta["EPC"]
    N = meta["N"]

    nc = bacc.Bacc("TRN2", target_bir_lowering=False, debug=False,
                   num_devices=n_cores, num_swdge_queues=4)

    def din(name, shape, d=dt.float32):
        return nc.dram_tensor(name, shape, d, kind="ExternalInput")

    h_d = din("h", [N, HID], dt.float32r)
    idx_d = din("idx16", [P, EPC // 16], dt.int16)
    dstl_d = din("dstl", [P, CE])
    degsrc_d = din("degsrc", [P, CE])
    degin_d = din("degin", [P, TN])
    gidl_d = din("gidl", [P, TN])
    gidl8_d = din("gidl8", [8, TN * P])
    wconv_d = din("w_conv", [HID, HID])
    w1_d = din("w1", [HID, HID])
    w2_d = din("w2", [HID, NCLS])
    bconv_d = din("bconv_bc", [P, HID])
    wg_d = din("wg_bc", [P, HID])
    bgate_d = din("bgate_col", [P, 1])
    b1_d = din("b1_bc8", [8, HID])
    b2_d = din("b2_bc8", [8, NCLS])
    iota128_d = din("iota128", [P, P])
    iotaK_d = din("iotaK", [P, K * P])
    iota8_d = din("iota8", [P, 8])
    iotac8_d = din("iota_col8", [8, 1])
    ident_d = din("ident", [P, P])

    att_o = nc.dram_tensor("att_out", [P, TN], dt.float32, kind="ExternalOutput")
    hg_o = nc.dram_tensor("hg_out", [8, HID], dt.float32, kind="ExternalOutput")
    sig_o = nc.dram_tensor("sig_out", [8, NCLS], dt.float32, kind="ExternalOutput")

    with tile.TileContext(nc) as tc:
        with (
            tc.tile_pool(name="const", bufs=1) as cpool,
            tc.tile_pool(name="meta", bufs=1) as mpool,
            tc.tile_pool(name="gather", bufs=3) as gpool,
            tc.tile_pool(name="oh", bufs=4) as ohpool,
            tc.tile_pool(name="work", bufs=3) as wpool,
            tc.tile_pool(name="psA", bufs=2, space="PSUM") as psA,
            tc.tile_pool(name="psB", bufs=2, space="PSUM") as psB,
            tc.tile_pool(name="psC", bufs=2, space="PSUM") as psC,
            tc.tile_pool(name="psAcc", bufs=1, space="PSUM") as psAcc,
        ):
            def load(pool, dram, shape, d=dt.float32):
                t = pool.tile(shape, d, tag=dram.name)
                nc.sync.dma_start(t[:], dram.ap())
                return t

            def load_w(dram, X, tag):
                t = cpool.tile([P, 2 * X], dt.float32, tag=tag)
                nc.sync.dma_start(t[:, 0:X], dram.ap()[0:P, :])
                nc.sync.dma_start(t[:, X:2 * X], dram.ap()[P:2 * P, :])
                return t

            idx16 = load(mpool, idx_d, [P, EPC // 16], dt.int16)
            wconv = load_w(wconv_d, HID, "wconv")
            w1 = load_w(w1_d, HID, "w1")
            w2 = load_w(w2_d, NCLS, "w2")
            bconv = load(cpool, bconv_d, [P, HID])
            wg = load(cpool, wg_d, [P, HID])
            bgate = load(cpool, bgate_d, [P, 1])
            b1b = load(cpool, b1_d, [8, HID])
            b2b = load(cpool, b2_d, [8, NCLS])
            iota128 = load(cpool, iota128_d, [P, P])
            iotaK = load(cpool, iotaK_d, [P, K * P])
            iota8 = load(cpool, iota8_d, [P, 8])
            iotac8 = load(cpool, iotac8_d, [8, 1])
            ident = load(cpool, ident_d, [P, P])
            dstl = load(mpool, dstl_d, [P, CE])
            degsrc = load(mpool, degsrc_d, [P, CE])
            degin = load(mpool, degin_d, [P, TN])
            gidl = load(mpool, gidl_d, [P, TN])
            gidl8 = load(mpool, gidl8_d, [8, TN * P])

            scol = mpool.tile([P, CE], dt.float32)
            nc.scalar.sqrt(scol[:], degsrc[:])
            nc.vector.reciprocal(scol[:], scol[:])
            rin = mpool.tile([P, TN], dt.float32)
            nc.scalar.sqrt(rin[:], degin[:])
            nc.vector.reciprocal(rin[:], rin[:])

            e_all = mpool.tile([P, TN], dt.float32)
            att_all = mpool.tile([P, TN], dt.float32)
            hg_ps = psAcc.tile([8, HID + 1], dt.float32, tag="hg")

            # dma_gather calls are capped at 1024 indices (SWDGE ring holds
            # 1024 descriptors); batch 8 chunks per call, spanning tiles.
            GB = 8
            msg_tiles = {}

            def get_msg(c):
                j = c // GB
                if j not in msg_tiles:
                    lo = j * GB
                    nch = min(GB, CE - lo)
                    mt = gpool.tile([P, nch, HID], dt.float32r, tag="msg")
                    nc.gpsimd.dma_gather(
                        mt[:], h_d.ap(),
                        idx16[:, lo * 8:(lo + nch) * 8],
                        nch * P, nch * P, HID,
                        queue_num=j % 4,
                    )
                    msg_tiles[j] = mt
                return msg_tiles[j][:, c % GB, :]

            for t in range(TN):
                agg_ps = psA.tile([P, HID], dt.float32, tag="agg")
                # one wide is_equal + one wide multiply build all K scaled
                # one-hot matrices at once (per-chunk tensor_scalar ops with
                # PTR scalars measured 8x slower under DMA contention)
                oh_all = ohpool.tile([P, K, P], dt.float32r, tag="oh")
                dv = dstl[:, t * K:(t + 1) * K].to_broadcast([P, K, P])
                sv = scol[:, t * K:(t + 1) * K].to_broadcast([P, K, P])
                nc.vector.tensor_tensor(
                    out=oh_all[:], in0=iotaK[:].rearrange(
                        "p (k q) -> p k q", k=K), in1=dv, op=op.is_equal)
                nc.vector.tensor_tensor(out=oh_all[:], in0=oh_all[:], in1=sv,
                                        op=op.mult)
                for k in range(K):
                    c = t * K + k
                    nc.tensor.matmul(agg_ps[:], lhsT=oh_all[:, k, :],
                                     rhs=get_msg(c),
                                     start=(k == 0), stop=(k == K - 1))
                agg_sb = wpool.tile([P, HID], dt.float32, tag="agg_sb")
                nc.vector.tensor_scalar(out=agg_sb[:], in0=agg_ps[:],
                                        scalar1=rin[:, t:t + 1], scalar2=None,
                                        op0=op.mult)
                tr_ps = psB.tile([P, HID], dt.float32, tag="tr")
                nc.tensor.transpose(tr_ps[:, 0:P], agg_sb[:, 0:P], ident[:])
                nc.tensor.transpose(tr_ps[:, P:HID], agg_sb[:, P:HID], ident[:])
                aggT = wpool.tile([P, HID], dt.float32, tag="aggT")
                nc.vector.tensor_copy(aggT[:], tr_ps[:])
                hr_ps = psC.tile([P, HID], dt.float32, tag="hr")
                for fh in range(2):
                    nc.tensor.matmul(hr_ps[:],
                                     lhsT=aggT[:, fh * P:(fh + 1) * P],
                                     rhs=wconv[:, fh * HID:(fh + 1) * HID],
                                     start=(fh == 0), stop=(fh == 1))
                hr = wpool.tile([P, HID + 1], dt.float32, tag="hr_sb")
                nc.vector.tensor_add(hr[:, 0:HID], hr_ps[:], bconv[:])
                nc.scalar.activation(hr[:, 0:HID], hr[:, 0:HID], act.Relu)
                nc.vector.memset(hr[:, HID:HID + 1], 1.0)
                ttr = wpool.tile([P, HID], dt.float32, tag="ttr")
                gcol = wpool.tile([P, 1], dt.float32, tag="gcol")
                nc.vector.scalar_tensor_tensor(
                    out=ttr[:], in0=hr[:, 0:HID], scalar=1.0, in1=wg[:],
                    op0=op.mult, op1=op.mult, accum_out=gcol[:],
                )
                nc.scalar.activation(e_all[:, t:t + 1], gcol[:], act.Exp,
                                     bias=bgate[:, 0:1])
                mg = wpool.tile([P, 8], dt.float32, tag="mg")
                nc.vector.tensor_scalar(
                    out=mg[:], in0=iota8[:],
                    scalar1=gidl[:, t:t + 1], scalar2=e_all[:, t:t + 1],
                    op0=op.is_equal, op1=op.mult,
                )
                nc.tensor.matmul(hg_ps[:], lhsT=mg[:], rhs=hr[:],
                                 start=(t == 0), stop=(t == TN - 1))

            dn = wpool.tile([8, 1], dt.float32, tag="dn")
            nc.vector.tensor_scalar(out=dn[:], in0=hg_ps[:, HID:HID + 1],
                                    scalar1=1e-30, scalar2=None, op0=op.max)
            recip = wpool.tile([8, 1], dt.float32, tag="recip")
            nc.vector.reciprocal(recip[:], dn[:])
            hg_sb = wpool.tile([8, HID], dt.float32, tag="hg_sb")
            nc.vector.tensor_scalar(out=hg_sb[:], in0=hg_ps[:, 0:HID],
                                    scalar1=recip[:, 0:1], scalar2=None,
                                    op0=op.mult)
            nc.sync.dma_start(hg_o.ap(), hg_sb[:])

            for t in range(TN):
                mgT = wpool.tile([8, P], dt.float32, tag="mgT")
                nc.vector.tensor_scalar(
                    out=mgT[:], in0=gidl8[:, t * P:(t + 1) * P],
                    scalar1=iotac8[:, 0:1], scalar2=None, op0=op.is_equal,
                )
                rp_ps = psB.tile([P, 1], dt.float32, tag="tr")
                nc.tensor.matmul(rp_ps[:], lhsT=mgT[:], rhs=recip[:],
                                 start=True, stop=True)
                nc.vector.tensor_mul(att_all[:, t:t + 1], e_all[:, t:t + 1],
                                     rp_ps[:])
            nc.sync.dma_start(att_o.ap(), att_all[:])

            def transpose2(src_sb, tag):
                tp = psB.tile([P, 16], dt.float32, tag="tr")
                nc.tensor.transpose(tp[:, 0:8], src_sb[:, 0:P], ident[0:8, 0:8])
                nc.tensor.transpose(tp[:, 8:16], src_sb[:, P:HID],
                                    ident[0:8, 0:8])
                sb = wpool.tile([P, 16], dt.float32, tag=tag)
                nc.vector.tensor_copy(sb[:], tp[:])
                return sb

            hgT = transpose2(hg_sb, "hgT")
            a2_ps = psA.tile([8, HID], dt.float32, tag="agg")
            for fh in range(2):
                nc.tensor.matmul(a2_ps[:], lhsT=hgT[:, fh * 8:(fh + 1) * 8],
                                 rhs=w1[:, fh * HID:(fh + 1) * HID],
                                 start=(fh == 0), stop=(fh == 1))
            a2 = wpool.tile([8, HID], dt.float32, tag="a2")
            nc.vector.tensor_add(a2[:], a2_ps[:], b1b[:])
            a2T = transpose2(a2, "a2T")
            a3_ps = psC.tile([8, NCLS], dt.float32, tag="hr")
            for fh in range(2):
                nc.tensor.matmul(a3_ps[:], lhsT=a2T[:, fh * 8:(fh + 1) * 8],
                                 rhs=w2[:, fh * NCLS:(fh + 1) * NCLS],
                                 start=(fh == 0), stop=(fh == 1))
            sig = wpool.tile([8, NCLS], dt.float32, tag="sig")
            nc.vector.tensor_add(sig[:], a3_ps[:], b2b[:])
            nc.scalar.activation(sig[:], sig[:], act.Sigmoid)
            nc.sync.dma_start(sig_o.ap(), sig[:])

    nc.compile()
    return nc


def _assemble(results, meta):
    n_cores = meta["n_cores"]
    N, G, gpc = meta["N"], meta["G"], meta["gpc"]
    slot_node = meta["slot_node"]
    out0 = np.zeros((G, NCLS), np.float32)
    hg = np.zeros((G, HID), np.float32)
    att = np.zeros((N, 1), np.float32)
    for c in range(n_cores):
        r = results[c]
        out0[c * gpc:(c + 1) * gpc] = r["sig_out"]
        hg[c * gpc:(c + 1) * gpc] = r["hg_out"]
        sn = slot_node[c]
        real = sn >= 0
        att[sn[real], 0] = r["att_out"].T[real]
    return out0, att, hg


TRACE = False          # set True (e.g. from test.py) to capture an NTFF profile
LAST_PERF = {}         # exec_time_ns etc. from the last traced run


def _install_axon_ntff_hook():
    """Best-effort: register the NTFF profile hook concourse expects under
    axon (the agent image's antenv lacks axon_hooks)."""
    import contextlib
    import ctypes
    import sys
    import types
    if 'antenv.axon_hooks' in sys.modules:
        return
    try:
        lib = ctypes.CDLL('/opt/axon/libaxon_pjrt.so')
        lib.axon_start_nrt_profile.argtypes = [ctypes.POINTER(ctypes.c_int64),
                                               ctypes.c_size_t]
        lib.axon_start_nrt_profile.restype = ctypes.c_int64
        lib.axon_stop_nrt_profile.argtypes = [ctypes.c_char_p]
        lib.axon_stop_nrt_profile.restype = ctypes.c_int64
    except (OSError, AttributeError):
        return

    @contextlib.contextmanager
    def _hook(output_dir, device_ids):
        import jax
        jax.devices()
        if device_ids:
            ids = (ctypes.c_int64 * len(device_ids))(*device_ids)
            rc = lib.axon_start_nrt_profile(ids, len(device_ids))
        else:
            rc = lib.axon_start_nrt_profile(None, 0)
        if rc != 0:
            raise RuntimeError(f"axon_start_nrt_profile rc={rc}")
        try:
            yield
        finally:
            lib.axon_stop_nrt_profile(str(output_dir).encode())

    mod = types.ModuleType('antenv.axon_hooks')
    mod.get_axon_ntff_profile_hook = lambda: _hook
    mod.set_axon_ntff_profile_hook = lambda h: None
    sys.modules['antenv.axon_hooks'] = mod


def kernel(**inputs):
    import sys
    if '/opt/trn_rl_repo' not in sys.path:
        sys.path.insert(0, '/opt/trn_rl_repo')
    from concourse import bass_utils
    from concourse.bass_utils import run_bass_kernel_spmd

    in_maps, meta = _prep(inputs, N_CORES)
    key = (meta["N"], meta["E"], meta["TN"], meta["K"])
    nc = _CACHE.get(key)
    if nc is None:
        nc = _build_nc(meta, N_CORES)
        _CACHE[key] = nc
    kwargs = {}
    if TRACE:
        _install_axon_ntff_hook()
        bass_utils.upload_artifacts = lambda d: d
        kwargs = dict(trace=True, trace_cores=list(range(N_CORES)))
    res = run_bass_kernel_spmd(nc, in_maps, core_ids=list(range(N_CORES)),
                               **kwargs)
    if TRACE:
        LAST_PERF.update(
            exec_time_ns=res.exec_time_ns,
            mean_exec_time_ns=res.mean_exec_time_ns,
            max_exec_time_core_id=res.max_exec_time_core_id,
            trace=(res.instructions_and_trace or (None, None))[1],
        )
    return _assemble(res.results, meta)


# revision 7
# speedup vs baseline: 1.3188x; 1.0870x over previous
"""Trainium2 Bass kernel: GCN layer + global-attention pooling + MLP head.

Contract: kernel(**inputs) takes the FULL (unsharded) numpy inputs of
reference.setup_inputs() and returns the FULL output tuple
(sigmoid_logits [G,2], att [N,1], hg [G,H]) — same structure as reference().

Sharding strategy (8 NeuronCores): nodes and their incoming edges are
partitioned by graph (graph_id is sorted; 8 graphs per core), so per-graph
softmax pooling is core-local.  h is replicated; each core gathers the
source rows for its own edges directly from DRAM with dma_gather and
scatter-adds them into PSUM via scaled one-hot matmuls on the PE.

Device math (W_conv commuted past the linear aggregation):
    agg_raw = segment_sum(rsqrt(deg_out)[src] * h[src] -> dst)
    hr      = relu((agg_raw * rsqrt(deg_in)) @ W_conv + b_conv)
    e       = exp(hr @ w_gate + b_gate)          (stable: gates are O(1))
    hg      = (one_hot(graph)ᵀ·e·[hr|1]) ;  att = e / denom[graph]
    out     = sigmoid((hg @ W1 + b1) @ W2 + b2)

Host preprocessing is integer graph restructuring only (bucketing edges by
destination tile, padding, degree counts); all float math runs on device.
"""

import heapq

import numpy as np

P = 128
HID = 256
NCLS = 2
N_CORES = 8

_CACHE = {}


# ---------------------------------------------------------------------------
# Host-side preprocessing (integer-only: sharding, bucketing, padding)
# ---------------------------------------------------------------------------

def _prep(inputs, n_cores=N_CORES):
    h = np.ascontiguousarray(np.asarray(inputs["h"], dtype=np.float32))
    src = np.asarray(inputs["src"]).astype(np.int64)
    dst = np.asarray(inputs["dst"]).astype(np.int64)
    gid = np.asarray(inputs["graph_id"]).astype(np.int64)
    N = h.shape[0]
    E = src.shape[0]
    G = max(int(gid.max()) + 1 if gid.size else 0, n_cores)
    gpc = (G + n_cores - 1) // n_cores
    assert gpc * n_cores == G and gpc == 8, (G, n_cores)

    deg_out = np.bincount(src, minlength=N)
    deg_in = np.bincount(dst, minlength=N)
    gcounts = np.bincount(gid, minlength=G)
    gstart = np.concatenate([[0], np.cumsum(gcounts)])

    core_lo = gstart[np.arange(n_cores) * gpc]
    core_hi = gstart[np.minimum((np.arange(n_cores) + 1) * gpc, G)]
    counts = core_hi - core_lo
    TN = int(-(-counts.max() // P))

    # Balanced node->(tile,partition) packing per core: place nodes (desc by
    # in-degree) into the least-loaded of TN bins with <128 nodes, so
    # per-tile edge counts are nearly equal -> minimal chunk count K.
    node_tile = np.full(N, -1, np.int64)
    node_part = np.full(N, -1, np.int64)
    slot_node = np.full((n_cores, TN, P), -1, np.int64)
    for c in range(n_cores):
        nodes = np.arange(core_lo[c], core_hi[c])
        order = nodes[np.argsort(-deg_in[nodes], kind="stable")]
        heap = [(0, 0, t) for t in range(TN)]
        heapq.heapify(heap)
        for n in order:
            load, cnt, t = heapq.heappop(heap)
            node_tile[n] = t
            node_part[n] = cnt
            slot_node[c, t, cnt] = n
            cnt += 1
            if cnt < P:
                heapq.heappush(heap, (load + int(deg_in[n]), cnt, t))

    node_core = np.searchsorted(core_hi, np.arange(N), side="right")
    e_core = node_core[dst]
    e_tile = node_tile[dst]
    tile_cnt = np.zeros((n_cores, TN), np.int64)
    np.add.at(tile_cnt, (e_core, e_tile), 1)
    K = int(-(-tile_cnt.max() // P))
    CE = TN * K
    EPC = CE * P

    srcpad = np.zeros((n_cores, EPC), np.int64)
    dstl = np.full((n_cores, EPC), 999.0, np.float32)
    degsrc = np.ones((n_cores, EPC), np.float32)
    order = np.lexsort((e_tile, e_core))
    eo = np.arange(E)[order]
    ec, et = e_core[order], e_tile[order]
    grp = ec * TN + et
    pos = np.arange(E) - np.concatenate(
        [[0], np.cumsum(np.bincount(grp, minlength=n_cores * TN))])[grp]
    slot = et * (K * P) + pos
    srcpad[ec, slot] = src[eo]
    dstl[ec, slot] = node_part[dst[eo]].astype(np.float32)
    degsrc[ec, slot] = np.maximum(deg_out[src[eo]], 1).astype(np.float32)

    # wrapped int16 gather indices: idx j -> partition j%16, col j//16,
    # replicated across the 8 groups of 16 partitions.
    assert N <= 32767
    idx16 = srcpad.reshape(n_cores, EPC // 16, 16).transpose(0, 2, 1)
    idx16 = np.tile(idx16.astype(np.int16), (1, 8, 1))
    dstl_t = dstl.reshape(n_cores, CE, P).transpose(0, 2, 1).copy()
    degsrc_t = degsrc.reshape(n_cores, CE, P).transpose(0, 2, 1).copy()

    degin_t = np.ones((n_cores, P, TN), np.float32)
    gidl_t = np.full((n_cores, P, TN), 999.0, np.float32)
    for c in range(n_cores):
        sn = slot_node[c]
        real = sn >= 0
        nn = sn[real]
        degin_t[c].T[real] = np.maximum(deg_in[nn], 1).astype(np.float32)
        gidl_t[c].T[real] = (gid[nn] - c * gpc).astype(np.float32)
    gidl8 = gidl_t.transpose(0, 2, 1).reshape(n_cores, 1, TN * P)
    gidl8 = np.broadcast_to(gidl8, (n_cores, 8, TN * P)).copy()

    W_conv = np.asarray(inputs["W_conv"], np.float32)
    b_conv = np.asarray(inputs["b_conv"], np.float32)
    w_gate = np.asarray(inputs["w_gate"], np.float32).reshape(-1)
    b_gate = np.asarray(inputs["b_gate"], np.float32).reshape(-1)
    W1 = np.asarray(inputs["W1"], np.float32)
    b1 = np.asarray(inputs["b1"], np.float32)
    W2 = np.asarray(inputs["W2"], np.float32)
    b2 = np.asarray(inputs["b2"], np.float32)

    consts = dict(
        iotaK=np.ascontiguousarray(np.broadcast_to(
            np.tile(np.arange(P, dtype=np.float32), K), (P, K * P))),
        w_conv=np.ascontiguousarray(W_conv),
        w1=np.ascontiguousarray(W1),
        w2=np.ascontiguousarray(W2),
        bconv_bc=np.broadcast_to(b_conv, (P, HID)).copy(),
        wg_bc=np.broadcast_to(w_gate, (P, HID)).copy(),
        bgate_col=np.full((P, 1), b_gate[0], np.float32),
        b1_bc8=np.broadcast_to(b1, (8, HID)).copy(),
        b2_bc8=np.broadcast_to(b2, (8, NCLS)).copy(),
        iota128=np.broadcast_to(np.arange(P, dtype=np.float32), (P, P)).copy(),
        iota8=np.broadcast_to(np.arange(8, dtype=np.float32), (P, 8)).copy(),
        iota_col8=np.arange(8, dtype=np.float32).reshape(8, 1),
        ident=np.eye(P, dtype=np.float32),
    )

    in_maps = []
    for c in range(n_cores):
        m = dict(consts)
        m["h"] = h
        m["idx16"] = np.ascontiguousarray(idx16[c])
        m["dstl"] = np.ascontiguousarray(dstl_t[c])
        m["degsrc"] = np.ascontiguousarray(degsrc_t[c])
        m["degin"] = np.ascontiguousarray(degin_t[c])
        m["gidl"] = np.ascontiguousarray(gidl_t[c])
        m["gidl8"] = np.ascontiguousarray(gidl8[c])
        in_maps.append(m)

    meta = dict(N=N, E=E, G=G, gpc=gpc, TN=TN, K=K, CE=CE, EPC=EPC,
                n_cores=n_cores, slot_node=slot_node)
    return in_maps, meta


# ---------------------------------------------------------------------------
# Device program
# ---------------------------------------------------------------------------

def _build_nc(meta, n_cores):
    import concourse.bacc as bacc
    import concourse.mybir as mybir
    import concourse.tile as tile

    dt = mybir.dt
    op = mybir.AluOpType
    act = mybir.ActivationFunctionType
    TN, K, CE, EPC = meta["TN"], meta["K"], meta["CE"], meta["EPC"]
    N = meta["N"]

    nc = bacc.Bacc("TRN2", target_bir_lowering=False, debug=False,
                   num_devices=n_cores, num_swdge_queues=4)

    def din(name, shape, d=dt.float32):
        return nc.dram_tensor(name, shape, d, kind="ExternalInput")

    h_d = din("h", [N, HID], dt.float32r)
    idx_d = din("idx16", [P, EPC // 16], dt.int16)
    dstl_d = din("dstl", [P, CE])
    degsrc_d = din("degsrc", [P, CE])
    degin_d = din("degin", [P, TN])
    gidl_d = din("gidl", [P, TN])
    gidl8_d = din("gidl8", [8, TN * P])
    wconv_d = din("w_conv", [HID, HID])
    w1_d = din("w1", [HID, HID])
    w2_d = din("w2", [HID, NCLS])
    bconv_d = din("bconv_bc", [P, HID])
    wg_d = din("wg_bc", [P, HID])
    bgate_d = din("bgate_col", [P, 1])
    b1_d = din("b1_bc8", [8, HID])
    b2_d = din("b2_bc8", [8, NCLS])
    iota128_d = din("iota128", [P, P])
    iotaK_d = din("iotaK", [P, K * P])
    iota8_d = din("iota8", [P, 8])
    iotac8_d = din("iota_col8", [8, 1])
    ident_d = din("ident", [P, P])

    att_o = nc.dram_tensor("att_out", [P, TN], dt.float32, kind="ExternalOutput")
    hg_o = nc.dram_tensor("hg_out", [8, HID], dt.float32, kind="ExternalOutput")
    sig_o = nc.dram_tensor("sig_out", [8, NCLS], dt.float32, kind="ExternalOutput")

    with tile.TileContext(nc) as tc:
        with (
            tc.tile_pool(name="const", bufs=1) as cpool,
            tc.tile_pool(name="meta", bufs=1) as mpool,
            tc.tile_pool(name="gather", bufs=5) as gpool,
            tc.tile_pool(name="oh", bufs=4) as ohpool,
            tc.tile_pool(name="work", bufs=3) as wpool,
            tc.tile_pool(name="psA", bufs=2, space="PSUM") as psA,
            tc.tile_pool(name="psB", bufs=2, space="PSUM") as psB,
            tc.tile_pool(name="psC", bufs=2, space="PSUM") as psC,
            tc.tile_pool(name="psAcc", bufs=1, space="PSUM") as psAcc,
        ):
            def load(pool, dram, shape, d=dt.float32):
                t = pool.tile(shape, d, tag=dram.name)
                nc.sync.dma_start(t[:], dram.ap())
                return t

            def load_w(dram, X, tag):
                t = cpool.tile([P, 2 * X], dt.float32, tag=tag)
                nc.sync.dma_start(t[:, 0:X], dram.ap()[0:P, :])
                nc.sync.dma_start(t[:, X:2 * X], dram.ap()[P:2 * P, :])
                return t

            idx16 = load(mpool, idx_d, [P, EPC // 16], dt.int16)
            wconv = load_w(wconv_d, HID, "wconv")
            w1 = load_w(w1_d, HID, "w1")
            w2 = load_w(w2_d, NCLS, "w2")
            bconv = load(cpool, bconv_d, [P, HID])
            wg = load(cpool, wg_d, [P, HID])
            bgate = load(cpool, bgate_d, [P, 1])
            b1b = load(cpool, b1_d, [8, HID])
            b2b = load(cpool, b2_d, [8, NCLS])
            iota128 = load(cpool, iota128_d, [P, P])
            iotaK = load(cpool, iotaK_d, [P, K * P])
            iota8 = load(cpool, iota8_d, [P, 8])
            iotac8 = load(cpool, iotac8_d, [8, 1])
            ident = load(cpool, ident_d, [P, P])
            dstl = load(mpool, dstl_d, [P, CE])
            degsrc = load(mpool, degsrc_d, [P, CE])
            degin = load(mpool, degin_d, [P, TN])
            gidl = load(mpool, gidl_d, [P, TN])
            gidl8 = load(mpool, gidl8_d, [8, TN * P])

            scol = mpool.tile([P, CE], dt.float32)
            nc.scalar.sqrt(scol[:], degsrc[:])
            nc.vector.reciprocal(scol[:], scol[:])
            rin = mpool.tile([P, TN], dt.float32)
            nc.scalar.sqrt(rin[:], degin[:])
            nc.vector.reciprocal(rin[:], rin[:])

            e_all = mpool.tile([P, TN], dt.float32)
            att_all = mpool.tile([P, TN], dt.float32)
            hg_ps = psAcc.tile([8, HID + 1], dt.float32, tag="hg")

            # dma_gather calls are capped at 1024 indices (SWDGE ring holds
            # 1024 descriptors); batch 8 chunks per call, spanning tiles.
            GB = 8
            msg_tiles = {}

            def get_msg(c):
                j = c // GB
                if j not in msg_tiles:
                    lo = j * GB
                    nch = min(GB, CE - lo)
                    mt = gpool.tile([P, nch, HID], dt.float32r, tag="msg")
                    nc.gpsimd.dma_gather(
                        mt[:], h_d.ap(),
                        idx16[:, lo * 8:(lo + nch) * 8],
                        nch * P, nch * P, HID,
                        queue_num=j % 4,
                    )
                    msg_tiles[j] = mt
                return msg_tiles[j][:, c % GB, :]

            for t in range(TN):
                agg_ps = psA.tile([P, HID], dt.float32, tag="agg")
                # one wide is_equal + one wide multiply build all K scaled
                # one-hot matrices at once (per-chunk tensor_scalar ops with
                # PTR scalars measured 8x slower under DMA contention)
                oh_all = ohpool.tile([P, K, P], dt.float32r, tag="oh")
                dv = dstl[:, t * K:(t + 1) * K].to_broadcast([P, K, P])
                sv = scol[:, t * K:(t + 1) * K].to_broadcast([P, K, P])
                nc.vector.tensor_tensor(
                    out=oh_all[:], in0=iotaK[:].rearrange(
                        "p (k q) -> p k q", k=K), in1=dv, op=op.is_equal)
                nc.vector.tensor_tensor(out=oh_all[:], in0=oh_all[:], in1=sv,
                                        op=op.mult)
                for k in range(K):
                    c = t * K + k
                    nc.tensor.matmul(agg_ps[:], lhsT=oh_all[:, k, :],
                                     rhs=get_msg(c),
                                     start=(k == 0), stop=(k == K - 1))
                agg_sb = wpool.tile([P, HID], dt.float32, tag="agg_sb")
                nc.vector.tensor_scalar(out=agg_sb[:], in0=agg_ps[:],
                                        scalar1=rin[:, t:t + 1], scalar2=None,
                                        op0=op.mult)
                tr_ps = psB.tile([P, HID], dt.float32, tag="tr")
                nc.tensor.transpose(tr_ps[:, 0:P], agg_sb[:, 0:P], ident[:])
                nc.tensor.transpose(tr_ps[:, P:HID], agg_sb[:, P:HID], ident[:])
                aggT = wpool.tile([P, HID], dt.float32, tag="aggT")
                nc.vector.tensor_copy(aggT[:], tr_ps[:])
                hr_ps = psC.tile([P, HID], dt.float32, tag="hr")
                for fh in range(2):
                    nc.tensor.matmul(hr_ps[:],
                                     lhsT=aggT[:, fh * P:(fh + 1) * P],
                                     rhs=wconv[:, fh * HID:(fh + 1) * HID],
                                     start=(fh == 0), stop=(fh == 1))
                hr = wpool.tile([P, HID + 1], dt.float32, tag="hr_sb")
                nc.vector.tensor_add(hr[:, 0:HID], hr_ps[:], bconv[:])
                nc.scalar.activation(hr[:, 0:HID], hr[:, 0:HID], act.Relu)
                nc.vector.memset(hr[:, HID:HID + 1], 1.0)
                ttr = wpool.tile([P, HID], dt.float32, tag="ttr")
                gcol = wpool.tile([P, 1], dt.float32, tag="gcol")
                nc.vector.scalar_tensor_tensor(
                    out=ttr[:], in0=hr[:, 0:HID], scalar=1.0, in1=wg[:],
                    op0=op.mult, op1=op.mult, accum_out=gcol[:],
                )
                nc.scalar.activation(e_all[:, t:t + 1], gcol[:], act.Exp,
                                     bias=bgate[:, 0:1])
                mg = wpool.tile([P, 8], dt.float32, tag="mg")
                nc.vector.tensor_scalar(
                    out=mg[:], in0=iota8[:],
                    scalar1=gidl[:, t:t + 1], scalar2=e_all[:, t:t + 1],
                    op0=op.is_equal, op1=op.mult,
                )
                nc.tensor.matmul(hg_ps[:], lhsT=mg[:], rhs=hr[:],
                                 start=(t == 0), stop=(t == TN - 1))

            dn = wpool.tile([8, 1], dt.float32, tag="dn")
            nc.vector.tensor_scalar(out=dn[:], in0=hg_ps[:, HID:HID + 1],
                                    scalar1=1e-30, scalar2=None, op0=op.max)
            recip = wpool.tile([8, 1], dt.float32, tag="recip")
            nc.vector.reciprocal(recip[:], dn[:])
            hg_sb = wpool.tile([8, HID], dt.float32, tag="hg_sb")
            nc.vector.tensor_scalar(out=hg_sb[:], in0=hg_ps[:, 0:HID],
                                    scalar1=recip[:, 0:1], scalar2=None,
                                    op0=op.mult)
            nc.sync.dma_start(hg_o.ap(), hg_sb[:])

            for t in range(TN):
                mgT = wpool.tile([8, P], dt.float32, tag="mgT")
                nc.vector.tensor_scalar(
                    out=mgT[:], in0=gidl8[:, t * P:(t + 1) * P],
                    scalar1=iotac8[:, 0:1], scalar2=None, op0=op.is_equal,
                )
                rp_ps = psB.tile([P, 1], dt.float32, tag="tr")
                nc.tensor.matmul(rp_ps[:], lhsT=mgT[:], rhs=recip[:],
                                 start=True, stop=True)
                nc.vector.tensor_mul(att_all[:, t:t + 1], e_all[:, t:t + 1],
                                     rp_ps[:])
            nc.sync.dma_start(att_o.ap(), att_all[:])

            def transpose2(src_sb, tag):
                tp = psB.tile([P, 16], dt.float32, tag="tr")
                nc.tensor.transpose(tp[:, 0:8], src_sb[:, 0:P], ident[0:8, 0:8])
                nc.tensor.transpose(tp[:, 8:16], src_sb[:, P:HID],
                                    ident[0:8, 0:8])
                sb = wpool.tile([P, 16], dt.float32, tag=tag)
                nc.vector.tensor_copy(sb[:], tp[:])
                return sb

            hgT = transpose2(hg_sb, "hgT")
            a2_ps = psA.tile([8, HID], dt.float32, tag="agg")
            for fh in range(2):
                nc.tensor.matmul(a2_ps[:], lhsT=hgT[:, fh * 8:(fh + 1) * 8],
                                 rhs=w1[:, fh * HID:(fh + 1) * HID],
                                 start=(fh == 0), stop=(fh == 1))
            a2 = wpool.tile([8, HID], dt.float32, tag="a2")
            nc.vector.tensor_add(a2[:], a2_ps[:], b1b[:])
            a2T = transpose2(a2, "a2T")
            a3_ps = psC.tile([8, NCLS], dt.float32, tag="hr")
            for fh in range(2):
                nc.tensor.matmul(a3_ps[:], lhsT=a2T[:, fh * 8:(fh + 1) * 8],
                                 rhs=w2[:, fh * NCLS:(fh + 1) * NCLS],
                                 start=(fh == 0), stop=(fh == 1))
            sig = wpool.tile([8, NCLS], dt.float32, tag="sig")
            nc.vector.tensor_add(sig[:], a3_ps[:], b2b[:])
            nc.scalar.activation(sig[:], sig[:], act.Sigmoid)
            nc.sync.dma_start(sig_o.ap(), sig[:])

    nc.compile()
    return nc


def _assemble(results, meta):
    n_cores = meta["n_cores"]
    N, G, gpc = meta["N"], meta["G"], meta["gpc"]
    slot_node = meta["slot_node"]
    out0 = np.zeros((G, NCLS), np.float32)
    hg = np.zeros((G, HID), np.float32)
    att = np.zeros((N, 1), np.float32)
    for c in range(n_cores):
        r = results[c]
        out0[c * gpc:(c + 1) * gpc] = r["sig_out"]
        hg[c * gpc:(c + 1) * gpc] = r["hg_out"]
        sn = slot_node[c]
        real = sn >= 0
        att[sn[real], 0] = r["att_out"].T[real]
    return out0, att, hg


TRACE = False          # set True (e.g. from test.py) to capture an NTFF profile
LAST_PERF = {}         # exec_time_ns etc. from the last traced run


def _install_axon_ntff_hook():
    """Best-effort: register the NTFF profile hook concourse expects under
    axon (the agent image's antenv lacks axon_hooks)."""
    import contextlib
    import ctypes
    import sys
    import types
    if 'antenv.axon_hooks' in sys.modules:
        return
    try:
        lib = ctypes.CDLL('/opt/axon/libaxon_pjrt.so')
        lib.axon_start_nrt_profile.argtypes = [ctypes.POINTER(ctypes.c_int64),
                                               ctypes.c_size_t]
        lib.axon_start_nrt_profile.restype = ctypes.c_int64
        lib.axon_stop_nrt_profile.argtypes = [ctypes.c_char_p]
        lib.axon_stop_nrt_profile.restype = ctypes.c_int64
    except (OSError, AttributeError):
        return

    @contextlib.contextmanager
    def _hook(output_dir, device_ids):
        import jax
        jax.devices()
        if device_ids:
            ids = (ctypes.c_int64 * len(device_ids))(*device_ids)
            rc = lib.axon_start_nrt_profile(ids, len(device_ids))
        else:
            rc = lib.axon_start_nrt_profile(None, 0)
        if rc != 0:
            raise RuntimeError(f"axon_start_nrt_profile rc={rc}")
        try:
            yield
        finally:
            lib.axon_stop_nrt_profile(str(output_dir).encode())

    mod = types.ModuleType('antenv.axon_hooks')
    mod.get_axon_ntff_profile_hook = lambda: _hook
    mod.set_axon_ntff_profile_hook = lambda h: None
    sys.modules['antenv.axon_hooks'] = mod


def kernel(**inputs):
    import sys
    if '/opt/trn_rl_repo' not in sys.path:
        sys.path.insert(0, '/opt/trn_rl_repo')
    from concourse import bass_utils
    from concourse.bass_utils import run_bass_kernel_spmd

    in_maps, meta = _prep(inputs, N_CORES)
    key = (meta["N"], meta["E"], meta["TN"], meta["K"])
    nc = _CACHE.get(key)
    if nc is None:
        nc = _build_nc(meta, N_CORES)
        _CACHE[key] = nc
    kwargs = {}
    if TRACE:
        _install_axon_ntff_hook()
        bass_utils.upload_artifacts = lambda d: d
        kwargs = dict(trace=True, trace_cores=list(range(N_CORES)))
    res = run_bass_kernel_spmd(nc, in_maps, core_ids=list(range(N_CORES)),
                               **kwargs)
    if TRACE:
        LAST_PERF.update(
            exec_time_ns=res.exec_time_ns,
            mean_exec_time_ns=res.mean_exec_time_ns,
            max_exec_time_core_id=res.max_exec_time_core_id,
            trace=(res.instructions_and_trace or (None, None))[1],
        )
    return _assemble(res.results, meta)
